# revision 13
# baseline (speedup 1.0000x reference)
"""Multi-head causal attention (B=2, S=2048, D=1024, H=16) on 8 TRN2 NeuronCores.

Sharding: (batch, head-group). Core c handles batch c//4 and heads
[4*(c%4) .. 4*(c%4)+3]:
  - Wq/Wk/Wv column-sliced [1024, 256] per core -> per-core q,k,v (4 heads)
  - causal attention for the 4 local heads (scoresT layout)
  - Wo row-sliced [256, 1024] -> bf16 partial output [2048, 1024] per core
  - host sums the 4 partials per batch (+bo) = exact all-reduce

Per-core work is identical to head-only sharding, but input DMA halves (one
batch of xT) and the partial output halves, so the kernel start/end stalls
shrink and there is a single fully-pipelined batch stream.

Pipeline: for each 512-row chunk jc, project q/k/v for that chunk, then run
attention i-chunk ic=jc (which only needs k/v up to chunk jc). Scores are
computed transposed (scoresT[j, i] = k_j . q_i); exp runs on pairs of j-tiles
(one activation over a 2-bank PSUM region) to halve activation instruction
overhead; the ctx matmul consumes at directly with V as stationary, with a
ones-column appended to V producing the softmax denominator for free. The
per-i reciprocal is broadcast to head rows via a tiny [33,128] matmul (exact
bf16 hi/lo split). Softmax skips max-subtraction: scores/8 ~ N(0,0.4), exp
cannot overflow.

PROJ_FP8: q/k/v projections run in fp8 e4m3 DoubleRow (dual-pump) mode with
weights pre-scaled by 64 on host (so they land in e4m3 normal range); the
64x factor on q,k cancels in softmax via the activation scale, and the 64x
on v is folded into the reciprocal-broadcast E matrix (1/64 entries).
"""

import numpy as np

B, S, D = 2, 2048, 1024
H, HD = 16, 64
NCORES = 8
HLOC = 4                 # heads per core
DLOC = HLOC * HD         # local qkv width = 256
SB = S                   # rows per core (one batch)
IC = SB // 512           # 4 i-chunks of 512
JT = SB // 128           # 16 j-tiles of 128
KT = D // 128            # 8 contraction tiles for projections

PROJ_FP8 = False          # fp8 DoubleRow q/k/v projections
WSCALE = 64.0            # host pre-scale on Wq/Wk/Wv for fp8 range

_CACHE = {}


def _install_ntff_hook():
    import sys, types
    if "antenv.axon_hooks" in sys.modules:
        return
    mod = types.ModuleType("antenv.axon_hooks")
    mod._hook = None
    mod.set_axon_ntff_profile_hook = lambda h: setattr(mod, "_hook", h)
    mod.get_axon_ntff_profile_hook = lambda: mod._hook
    sys.modules["antenv.axon_hooks"] = mod
    import antenv
    antenv.axon_hooks = mod
    try:
        from trn_agent_boot.trn_boot import _ntff_profile_via_ctypes
        mod.set_axon_ntff_profile_hook(
            _ntff_profile_via_ctypes("/opt/axon/libaxon_pjrt.so"))
    except Exception:
        pass


def _build():
    import concourse.bass as bass
    import concourse.tile as tile
    from concourse import bacc, mybir

    f32 = mybir.dt.float32
    bf16 = mybir.dt.bfloat16
    f8 = mybir.dt.float8e4
    pdt = f8 if PROJ_FP8 else bf16      # projection operand dtype
    EXP = mybir.ActivationFunctionType.Exp
    DR = mybir.MatmulPerfMode.DoubleRow if PROJ_FP8 else None
    escale = (1.0 / WSCALE) if PROJ_FP8 else 1.0      # folds v's 64x away
    ascale = 0.125 / (WSCALE * WSCALE) if PROJ_FP8 else 0.125

    nc = bacc.Bacc("TRN2", target_bir_lowering=False, debug=False,
                   num_devices=NCORES)
    # xt: kt-major [128, KT*2048]; col = kt*SB + r
    xt_d = nc.dram_tensor("xt", [128, KT * SB], pdt, kind="ExternalInput").ap()
    # wq/wk/wv: kt-major [128, KT*256]
    wq_d = nc.dram_tensor("wq", [128, KT * DLOC], pdt, kind="ExternalInput").ap()
    wk_d = nc.dram_tensor("wk", [128, KT * DLOC], pdt, kind="ExternalInput").ap()
    wv_d = nc.dram_tensor("wv", [128, KT * DLOC], pdt, kind="ExternalInput").ap()
    # wo: row-blocked [128, 2*1024]
    wo_d = nc.dram_tensor("wo", [128, 2 * D], bf16, kind="ExternalInput").ap()
    out_d = nc.dram_tensor("out", [SB, D], bf16, kind="ExternalOutput").ap()

    with tile.TileContext(nc) as tc:
        with tc.tile_pool(name="const", bufs=1) as cpool, \
             tc.tile_pool(name="w", bufs=1) as wpool, \
             tc.tile_pool(name="xt", bufs=1) as xtpool, \
             tc.tile_pool(name="qk", bufs=1) as qkpool, \
             tc.tile_pool(name="ve", bufs=1) as vepool, \
             tc.tile_pool(name="at", bufs=4) as atpool, \
             tc.tile_pool(name="cx", bufs=2) as cxpool, \
             tc.tile_pool(name="dn", bufs=2) as dnpool, \
             tc.tile_pool(name="sm", bufs=2) as smpool, \
             tc.tile_pool(name="ot", bufs=4) as otpool, \
             tc.tile_pool(name="ps", bufs=2, space="PSUM") as ps_s, \
             tc.tile_pool(name="pc", bufs=2, space="PSUM") as ps_c, \
             tc.tile_pool(name="pm", bufs=2, space="PSUM") as ps_m:

            # ---- constants ----
            # E: bcast matrix, row 0 -> out rows 0:64, row 32 -> rows 64:128
            # (escale folds the fp8 WSCALE on v back out)
            e_f = cpool.tile([64, 128], f32, tag="e_f")
            nc.gpsimd.memset(e_f[:], 0.0)
            nc.gpsimd.affine_select(
                out=e_f[0:32, :], in_=e_f[0:32, :],
                compare_op=mybir.AluOpType.is_ge,
                fill=escale, base=-64, pattern=[[1, 128]], channel_multiplier=64)
            nc.gpsimd.affine_select(
                out=e_f[32:64, :], in_=e_f[32:64, :],
                compare_op=mybir.AluOpType.is_ge,
                fill=escale, base=63, pattern=[[-1, 128]], channel_multiplier=64)
            emat = cpool.tile([33, 128], bf16, tag="emat")
            nc.vector.tensor_copy(emat[:], e_f[0:33, :])

            # ---- weights ----
            wq_sb = wpool.tile([128, KT * DLOC], pdt, tag="wq")
            wk_sb = wpool.tile([128, KT * DLOC], pdt, tag="wk")
            wv_sb = wpool.tile([128, KT * DLOC], pdt, tag="wv")
            wo_sb = wpool.tile([128, 2 * D], bf16, tag="wo")
            nc.sync.dma_start(wq_sb[:], wq_d[:])
            nc.sync.dma_start(wk_sb[:], wk_d[:])
            nc.sync.dma_start(wv_sb[:], wv_d[:])

            # ---- xT (kt-major single tile), chunked DMA so compute can
            # start after the first chunk ----
            xt_sb = xtpool.tile([128, KT * SB], pdt, tag="xt")
            for jc in range(IC):
                c0 = jc * 512
                for kt in range(KT):
                    nc.sync.dma_start(
                        xt_sb[:, kt * SB + c0:kt * SB + c0 + 512],
                        xt_d[:, kt * SB + c0:kt * SB + c0 + 512])
                if jc == 0:
                    nc.sync.dma_start(wo_sb[:], wo_d[:])

            def xts(kt, a, b):
                return xt_sb[:, kt * SB + a:kt * SB + b]

            # persistent q/k (scoresT layout) and v-ext tiles
            # qt[g]/kt_t[g]: [128, SB], rows h*64 hold head hp=g, h
            qt = [qkpool.tile([128, SB], bf16, tag=f"q{g}", name=f"qt{g}")
                  for g in range(2)]
            kt_t = [qkpool.tile([128, SB], bf16, tag=f"k{g}", name=f"ktt{g}")
                    for g in range(2)]
            # ve: [128, 4*1040]; col h*1040 + jt*65 + d, d=64 is the ones col
            ve = vepool.tile([128, HLOC * 65 * JT], bf16, tag="ve")
            vev = ve[:].rearrange("p (h j c) -> p h j c", h=HLOC, c=65)
            for h in range(HLOC):
                nc.gpsimd.memset(vev[:, h, :, 64], 1.0)

            ctxR = {}

            for jc in range(IC):
                c0 = jc * 512
                # ---- q/k projections for i-chunk jc ----
                for w_sb, dest in ((wq_sb, qt), (wk_sb, kt_t)):
                    for g in range(2):
                        P = ps_m.tile([128, 512], f32, tag="m")
                        if PROJ_FP8:
                            for kp in range(KT // 2):
                                st = w_sb[:, 2 * kp * DLOC:(2 * kp + 2) * DLOC]
                                st = st.rearrange(
                                    "p (two f) -> p two f",
                                    two=2)[:, :, g * 128:(g + 1) * 128]
                                mv = xt_sb[:, 2 * kp * SB:(2 * kp + 2) * SB]
                                mv = mv.rearrange("p (two f) -> p two f",
                                                  two=2)[:, :, c0:c0 + 512]
                                nc.tensor.matmul(
                                    P[:], st, mv, start=(kp == 0),
                                    stop=(kp == KT // 2 - 1), perf_mode=DR)
                        else:
                            for kt in range(KT):
                                nc.tensor.matmul(
                                    P[:],
                                    w_sb[:, kt * DLOC + g * 128:
                                         kt * DLOC + (g + 1) * 128],
                                    xts(kt, c0, c0 + 512),
                                    start=(kt == 0), stop=(kt == KT - 1))
                        nc.vector.tensor_copy(dest[g][:, c0:c0 + 512], P[:])

                # ---- v projection for j-tiles of this chunk ----
                for jp in range(2):
                    Pv = ps_m.tile([128, 512], f32, tag="m")
                    for sub in range(2):
                        jt = jc * 4 + jp * 2 + sub
                        if PROJ_FP8:
                            for kp in range(KT // 2):
                                st = xt_sb[:, 2 * kp * SB:(2 * kp + 2) * SB]
                                st = st.rearrange(
                                    "p (two f) -> p two f",
                                    two=2)[:, :, jt * 128:(jt + 1) * 128]
                                mv = wv_sb[:, 2 * kp * DLOC:(2 * kp + 2) * DLOC]
                                mv = mv.rearrange("p (two f) -> p two f", two=2)
                                nc.tensor.matmul(
                                    Pv[:, sub * 256:(sub + 1) * 256], st, mv,
                                    start=(kp == 0), stop=(kp == KT // 2 - 1),
                                    perf_mode=DR, skip_group_check=True)
                        else:
                            for kt in range(KT):
                                nc.tensor.matmul(
                                    Pv[:, sub * 256:(sub + 1) * 256],
                                    xts(kt, jt * 128, (jt + 1) * 128),
                                    wv_sb[:, kt * DLOC:(kt + 1) * DLOC],
                                    start=(kt == 0), stop=(kt == KT - 1),
                                    skip_group_check=True)
                    for sub in range(2):
                        jt = jc * 4 + jp * 2 + sub
                        src = Pv[:, sub * 256:(sub + 1) * 256].rearrange(
                            "p (h d) -> p h d", h=HLOC)
                        nc.vector.tensor_copy(vev[:, :, jt, 0:64], src)

                # ---- attention i-chunk ic = jc ----
                npair = 2 * jc + 2
                for hp in range(2):
                    den = dnpool.tile([33, 512], f32, tag="den")
                    nc.gpsimd.memset(den[:], 1.0)
                    ctxT = cxpool.tile([128, 512], f32, tag=f"ct{hp}")
                    for h in range(2):
                        Pc = ps_c.tile([65, 512], f32, tag="ctx")
                        hh = hp * 2 + h
                        pend = []   # deferred ctx matmuls (software pipeline)

                        def flush():
                            for (at_, jt_, e0_, sub_) in pend:
                                nc.tensor.matmul(
                                    Pc[:, e0_:512],
                                    vev[:, hh, jt_, :],
                                    at_[:, sub_ * 512 + e0_:(sub_ + 1) * 512],
                                    start=(jt_ == 0),
                                    stop=(jt_ == 4 * jc + 3),
                                    skip_group_check=True)
                            pend.clear()

                        for p in range(npair):
                            Ps = ps_s.tile([128, 1024], f32, tag="s")
                            info = []
                            for sub in range(2):
                                jt = 2 * p + sub
                                kb = jt - 4 * jc
                                e0 = 0 if kb < 0 else 128 * kb
                                nc.tensor.matmul(
                                    Ps[:, sub * 512 + e0:(sub + 1) * 512],
                                    kt_t[hp][h * 64:(h + 1) * 64,
                                             jt * 128:(jt + 1) * 128],
                                    qt[hp][h * 64:(h + 1) * 64,
                                           c0 + e0:c0 + 512],
                                    start=True, stop=True)
                                info.append((jt, kb, e0))
                            at = atpool.tile([128, 1024], bf16, tag="at")
                            e0L = info[0][2]
                            nc.scalar.activation(
                                at[:, e0L:1024], Ps[:, e0L:1024], EXP,
                                scale=ascale)
                            for sub in range(2):
                                jt, kb, e0 = info[sub]
                                if kb >= 0:
                                    nc.gpsimd.affine_select(
                                        out=at[:, sub * 512 + e0:
                                               sub * 512 + e0 + 128],
                                        in_=at[:, sub * 512 + e0:
                                               sub * 512 + e0 + 128],
                                        compare_op=mybir.AluOpType.is_ge,
                                        fill=0.0, base=0, pattern=[[1, 128]],
                                        channel_multiplier=-1)
                            flush()
                            for sub in range(2):
                                jt, kb, e0 = info[sub]
                                pend.append((at, jt, e0, sub))
                        flush()
                        # free the Pc bank quickly: den row + ctx rows out
                        # via the pool engine (vector stays light)
                        nc.vector.tensor_copy(den[h * 32:h * 32 + 1, :],
                                              Pc[64:65, :])
                        nc.vector.tensor_copy(
                            ctxT[h * 64:(h + 1) * 64, :], Pc[0:64, :])
                    # reciprocal of denominators (rows 0 and 32 carry data)
                    scr = smpool.tile([33, 512], f32, tag="scr")
                    rr = smpool.tile([33, 512], f32, tag="rr")
                    nc.vector.reciprocal_approx_accurate(rr[:], den[:], scr[:])
                    rhi = smpool.tile([33, 512], bf16, tag="rhi")
                    nc.vector.tensor_copy(rhi[:], rr[:])
                    rlo = smpool.tile([33, 512], bf16, tag="rlo")
                    nc.vector.tensor_sub(rlo[:], rr[:], rhi[:])
                    cx = cxpool.tile([128, 512], bf16, tag=f"cx{hp}",
                                     name=f"cx{hp}")
                    ctxR[hp] = (cx, ctxT, rhi, rlo)
                # Pb broadcast + ctx normalize, deferred so the vector chain
                # overlaps the other head-pair's score stream
                for hp in range(2):
                    cx, ctxT, rhi, rlo = ctxR[hp]
                    Pb = ps_m.tile([128, 512], f32, tag="m")
                    nc.tensor.matmul(Pb[:], emat[:], rhi[:], start=True,
                                     stop=False, skip_group_check=True)
                    nc.tensor.matmul(Pb[:], emat[:], rlo[:], start=False,
                                     stop=True, skip_group_check=True)
                    nc.vector.tensor_mul(cx[:], ctxT[:], Pb[:])

                # ---- output projection for this i-chunk ----
                for isl in range(4):
                    ot = otpool.tile([128, D], bf16, tag="ot")
                    for nk in range(2):
                        Po = ps_m.tile([128, 512], f32, tag="m")
                        nc.tensor.matmul(
                            Po[:], ctxR[0][0][:, isl * 128:(isl + 1) * 128],
                            wo_sb[:, nk * 512:(nk + 1) * 512],
                            start=True, stop=False, skip_group_check=True)
                        nc.tensor.matmul(
                            Po[:], ctxR[1][0][:, isl * 128:(isl + 1) * 128],
                            wo_sb[:, D + nk * 512:D + (nk + 1) * 512],
                            start=False, stop=True, skip_group_check=True)
                        nc.vector.tensor_copy(
                            ot[:, nk * 512:(nk + 1) * 512], Po[:])
                    nc.sync.dma_start(
                        out_d[c0 + isl * 128:c0 + (isl + 1) * 128, :], ot[:])

    nc.compile()
    return nc


def _get_nc():
    if "nc" not in _CACHE:
        _install_ntff_hook()
        _CACHE["nc"] = _build()
    return _CACHE["nc"]


def _run(inputs, trace=False):
    from concourse.bass_utils import run_bass_kernel_spmd
    import ml_dtypes

    nc = _get_nc()
    x = np.asarray(inputs["x"], dtype=np.float32)
    Wq = np.asarray(inputs["Wq"], dtype=np.float32)
    Wk = np.asarray(inputs["Wk"], dtype=np.float32)
    Wv = np.asarray(inputs["Wv"], dtype=np.float32)
    Wo = np.asarray(inputs["Wo"], dtype=np.float32)
    bo = np.asarray(inputs["bo"], dtype=np.float32)

    pnp = ml_dtypes.float8_e4m3 if PROJ_FP8 else ml_dtypes.bfloat16
    ws = WSCALE if PROJ_FP8 else 1.0

    def kt_major(a, w):
        # [1024, w] -> [128, KT*w] kt-major per partition
        return np.ascontiguousarray(
            a.reshape(KT, 128, w).transpose(1, 0, 2).reshape(128, KT * w))

    in_maps = []
    for c in range(NCORES):
        b, hg = divmod(c, 4)
        sl = slice(hg * DLOC, (hg + 1) * DLOC)
        xtb = kt_major(np.ascontiguousarray(x[b].T), SB).astype(pnp)
        in_maps.append({
            "xt": xtb,
            "wq": kt_major(Wq[:, sl] * ws, DLOC).astype(pnp),
            "wk": kt_major(Wk[:, sl] * ws, DLOC).astype(pnp),
            "wv": kt_major(Wv[:, sl] * ws, DLOC).astype(pnp),
            "wo": np.ascontiguousarray(
                Wo[sl, :].reshape(2, 128, D).transpose(1, 0, 2)
                .reshape(128, 2 * D)).astype(ml_dtypes.bfloat16),
        })
    res = run_bass_kernel_spmd(nc, in_maps, core_ids=list(range(NCORES)),
                               trace=trace)
    out = np.zeros((B, SB, D), dtype=np.float32)
    for c in range(NCORES):
        b = c // 4
        out[b] += res.results[c]["out"].astype(np.float32)
    out += bo[None, None, :]
    return out, res


def kernel(**inputs):
    out, _ = _run(inputs, trace=False)
    return out


# revision 14
# speedup vs baseline: 1.2185x; 1.2185x over previous
"""Multi-head causal attention (B=2, S=2048, D=1024, H=16) on 8 TRN2 NeuronCores.

Sharding: (batch, head-group). Core c handles batch c//4 and heads
[4*(c%4) .. 4*(c%4)+3]:
  - Wq/Wk/Wv column-sliced [1024, 256] per core -> per-core q,k,v (4 heads)
  - causal attention for the 4 local heads (scoresT layout)
  - Wo row-sliced [256, 1024] -> bf16 partial output [2048, 1024] per core
  - host sums the 4 partials per batch (+bo) = exact all-reduce

Versus head-only sharding this halves the input DMA (one batch of xT) and
halves the partial-output DMA (written bf16), and leaves a single batch
stream that pipelines cleanly.

Schedule: the kernel is a sequence of attention i-chunks (512 rows). The
q/k/v projection for chunk jc+1 and the output projection for chunk jc-1 are
emitted as background tensor groups interleaved between attention score
pairs of chunk jc, so the tensor engine never idles while the scalar engine
works through the exp stream (and vice versa).

Scores are computed transposed (scoresT[j, i] = k_j . q_i); exp runs on
pairs of j-tiles (one activation over a 2-bank PSUM region) to halve
activation instruction overhead; the attn@V matmul consumes at directly as
the moving operand with V stationary, and a ones-column appended to V
yields the softmax denominator for free (row 64 of the ctx PSUM). The
per-i reciprocal is broadcast to head rows via a tiny [33,128] matmul
(exact bf16 hi/lo split). Softmax skips max-subtraction: scores/8 ~
N(0,0.4), exp cannot overflow. All matmuls run bf16 with f32 PSUM.
"""

import numpy as np

B, S, D = 2, 2048, 1024
H, HD = 16, 64
NCORES = 8
HLOC = 4                 # heads per core
DLOC = HLOC * HD         # local qkv width = 256
SB = S                   # rows per core (one batch)
IC = SB // 512           # 4 i-chunks of 512
JT = SB // 128           # 16 j-tiles of 128
KT = D // 128            # 8 contraction tiles for projections

_CACHE = {}


def _install_ntff_hook():
    import sys, types
    if "antenv.axon_hooks" in sys.modules:
        return
    mod = types.ModuleType("antenv.axon_hooks")
    mod._hook = None
    mod.set_axon_ntff_profile_hook = lambda h: setattr(mod, "_hook", h)
    mod.get_axon_ntff_profile_hook = lambda: mod._hook
    sys.modules["antenv.axon_hooks"] = mod
    import antenv
    antenv.axon_hooks = mod
    try:
        from trn_agent_boot.trn_boot import _ntff_profile_via_ctypes
        mod.set_axon_ntff_profile_hook(
            _ntff_profile_via_ctypes("/opt/axon/libaxon_pjrt.so"))
    except Exception:
        pass


def _build():
    import concourse.bass as bass
    import concourse.tile as tile
    from concourse import bacc, mybir

    f32 = mybir.dt.float32
    bf16 = mybir.dt.bfloat16
    EXP = mybir.ActivationFunctionType.Exp

    nc = bacc.Bacc("TRN2", target_bir_lowering=False, debug=False,
                   num_devices=NCORES)
    # xt: jc-major [128, IC*4096]; col = jc*4096 + kt*512 + r  (r in-chunk)
    xt_d = nc.dram_tensor("xt", [128, KT * SB], bf16, kind="ExternalInput").ap()
    # wq/wk/wv: kt-major [128, KT*256]
    wq_d = nc.dram_tensor("wq", [128, KT * DLOC], bf16, kind="ExternalInput").ap()
    wk_d = nc.dram_tensor("wk", [128, KT * DLOC], bf16, kind="ExternalInput").ap()
    wv_d = nc.dram_tensor("wv", [128, KT * DLOC], bf16, kind="ExternalInput").ap()
    # wo: row-blocked [128, 2*1024]
    wo_d = nc.dram_tensor("wo", [128, 2 * D], bf16, kind="ExternalInput").ap()
    out_d = nc.dram_tensor("out", [SB, D], bf16, kind="ExternalOutput").ap()

    with tile.TileContext(nc) as tc:
        with tc.tile_pool(name="const", bufs=1) as cpool, \
             tc.tile_pool(name="w", bufs=1) as wpool, \
             tc.tile_pool(name="xt", bufs=1) as xtpool, \
             tc.tile_pool(name="qk", bufs=1) as qkpool, \
             tc.tile_pool(name="ve", bufs=1) as vepool, \
             tc.tile_pool(name="at", bufs=4) as atpool, \
             tc.tile_pool(name="cx", bufs=2) as cxpool, \
             tc.tile_pool(name="dn", bufs=2) as dnpool, \
             tc.tile_pool(name="sm", bufs=2) as smpool, \
             tc.tile_pool(name="ot", bufs=4) as otpool, \
             tc.tile_pool(name="ps", bufs=2, space="PSUM") as ps_s, \
             tc.tile_pool(name="pc", bufs=2, space="PSUM") as ps_c, \
             tc.tile_pool(name="pm", bufs=2, space="PSUM") as ps_m:

            # ---- constants ----
            # E: bcast matrix, row 0 -> out rows 0:64, row 32 -> rows 64:128
            e_f = cpool.tile([64, 128], f32, tag="e_f")
            nc.gpsimd.memset(e_f[:], 0.0)
            nc.gpsimd.affine_select(
                out=e_f[0:32, :], in_=e_f[0:32, :],
                compare_op=mybir.AluOpType.is_ge,
                fill=1.0, base=-64, pattern=[[1, 128]], channel_multiplier=64)
            nc.gpsimd.affine_select(
                out=e_f[32:64, :], in_=e_f[32:64, :],
                compare_op=mybir.AluOpType.is_ge,
                fill=1.0, base=63, pattern=[[-1, 128]], channel_multiplier=64)
            emat = cpool.tile([33, 128], bf16, tag="emat")
            nc.vector.tensor_copy(emat[:], e_f[0:33, :])

            # ---- weights + xt DMA (order chosen so chunk-0 compute can
            # start after ~2MB) ----
            wq_sb = wpool.tile([128, KT * DLOC], bf16, tag="wq")
            wk_sb = wpool.tile([128, KT * DLOC], bf16, tag="wk")
            wv_sb = wpool.tile([128, KT * DLOC], bf16, tag="wv")
            wo_sb = wpool.tile([128, 2 * D], bf16, tag="wo")
            xt_sb = xtpool.tile([128, KT * SB], bf16, tag="xt")
            nc.sync.dma_start(wq_sb[:], wq_d[:])
            nc.sync.dma_start(xt_sb[:, 0:4096], xt_d[:, 0:4096])
            nc.sync.dma_start(wk_sb[:], wk_d[:])
            nc.sync.dma_start(wv_sb[:], wv_d[:])
            nc.sync.dma_start(xt_sb[:, 4096:8192], xt_d[:, 4096:8192])
            nc.sync.dma_start(wo_sb[:], wo_d[:])
            for jc in range(2, IC):
                nc.sync.dma_start(xt_sb[:, jc * 4096:(jc + 1) * 4096],
                                  xt_d[:, jc * 4096:(jc + 1) * 4096])

            def xts(kt, a, b):
                jc, r = divmod(a, 512)
                off = jc * 4096 + kt * 512 + r
                return xt_sb[:, off:off + (b - a)]

            # persistent q/k (scoresT layout) and v-ext tiles
            qt = [qkpool.tile([128, SB], bf16, tag=f"q{g}", name=f"qt{g}")
                  for g in range(2)]
            kt_t = [qkpool.tile([128, SB], bf16, tag=f"k{g}", name=f"ktt{g}")
                    for g in range(2)]
            # ve: [128, 4*1040]; col h*1040 + jt*65 + d, d=64 is the ones col
            ve = vepool.tile([128, HLOC * 65 * JT], bf16, tag="ve")
            vev = ve[:].rearrange("p (h j c) -> p h j c", h=HLOC, c=65)
            for h in range(HLOC):
                nc.gpsimd.memset(vev[:, h, :, 64], 1.0)

            # ---- background tensor groups (emitted between score pairs) ---
            def qk_group(jc, w_sb, dest, g):
                def emit():
                    c0 = jc * 512
                    P = ps_m.tile([128, 512], f32, tag="m", name="Pqk")
                    for kt in range(KT):
                        nc.tensor.matmul(
                            P[:],
                            w_sb[:, kt * DLOC + g * 128:
                                 kt * DLOC + (g + 1) * 128],
                            xts(kt, c0, c0 + 512),
                            start=(kt == 0), stop=(kt == KT - 1))
                    nc.vector.tensor_copy(dest[g][:, c0:c0 + 512], P[:])
                return emit

            def v_group(jc, jp):
                def emit():
                    Pv = ps_m.tile([128, 512], f32, tag="m", name="Pv")
                    for sub in range(2):
                        jt = jc * 4 + jp * 2 + sub
                        for kt in range(KT):
                            nc.tensor.matmul(
                                Pv[:, sub * 256:(sub + 1) * 256],
                                xts(kt, jt * 128, (jt + 1) * 128),
                                wv_sb[:, kt * DLOC:(kt + 1) * DLOC],
                                start=(kt == 0), stop=(kt == KT - 1),
                                skip_group_check=True)
                    for sub in range(2):
                        jt = jc * 4 + jp * 2 + sub
                        src = Pv[:, sub * 256:(sub + 1) * 256].rearrange(
                            "p (h d) -> p h d", h=HLOC)
                        nc.vector.tensor_copy(vev[:, :, jt, 0:64], src)
                return emit

            def proj_groups(jc):
                gs = []
                for w_sb, dest in ((wq_sb, qt), (wk_sb, kt_t)):
                    for g in range(2):
                        gs.append(qk_group(jc, w_sb, dest, g))
                for jp in range(2):
                    gs.append(v_group(jc, jp))
                return gs

            def outproj_units(ic, cxs):
                us = []
                c0 = ic * 512
                for isl in range(4):
                    ot = otpool.tile([128, D], bf16, tag="ot", name="ot")

                    def unit(isl=isl, ot=ot):
                        for nk in range(2):
                            Po = ps_m.tile([128, 512], f32, tag="m", name="Po")
                            nc.tensor.matmul(
                                Po[:], cxs[0][:, isl * 128:(isl + 1) * 128],
                                wo_sb[:, nk * 512:(nk + 1) * 512],
                                start=True, stop=False, skip_group_check=True)
                            nc.tensor.matmul(
                                Po[:], cxs[1][:, isl * 128:(isl + 1) * 128],
                                wo_sb[:, D + nk * 512:D + (nk + 1) * 512],
                                start=False, stop=True, skip_group_check=True)
                            nc.vector.tensor_copy(
                                ot[:, nk * 512:(nk + 1) * 512], Po[:])
                        nc.sync.dma_start(
                            out_d[c0 + isl * 128:c0 + (isl + 1) * 128, :],
                            ot[:])
                    us.append(unit)
                return us

            # ---- attention i-chunk with background interleave ----
            def attention(jc, bg):
                c0 = jc * 512
                npair = 2 * jc + 2
                total_pairs = 4 * npair
                stride = max(1, total_pairs // max(1, len(bg)))
                state = {"pcount": 0}
                cxs = []
                chains = []
                for hp in range(2):
                    den = dnpool.tile([33, 512], f32, tag="den", name="den")
                    nc.gpsimd.memset(den[:], 1.0)
                    ctxT = cxpool.tile([128, 512], f32, tag=f"ct{hp}",
                                       name=f"ct{hp}")
                    for h in range(2):
                        Pc = ps_c.tile([65, 512], f32, tag="ctx", name="Pc")
                        hh = hp * 2 + h
                        pend = []

                        def flush(Pc=Pc, hh=hh, pend=pend):
                            for (at_, jt_, e0_, sub_) in pend:
                                nc.tensor.matmul(
                                    Pc[:, e0_:512],
                                    vev[:, hh, jt_, :],
                                    at_[:, sub_ * 512 + e0_:(sub_ + 1) * 512],
                                    start=(jt_ == 0),
                                    stop=(jt_ == 4 * jc + 3),
                                    skip_group_check=True)
                            pend.clear()

                        for p in range(npair):
                            Ps = ps_s.tile([128, 1024], f32, tag="s",
                                           name="Ps")
                            info = []
                            for sub in range(2):
                                jt = 2 * p + sub
                                kb = jt - 4 * jc
                                e0 = 0 if kb < 0 else 128 * kb
                                nc.tensor.matmul(
                                    Ps[:, sub * 512 + e0:(sub + 1) * 512],
                                    kt_t[hp][h * 64:(h + 1) * 64,
                                             jt * 128:(jt + 1) * 128],
                                    qt[hp][h * 64:(h + 1) * 64,
                                           c0 + e0:c0 + 512],
                                    start=True, stop=True,
                                    skip_group_check=True)
                                info.append((jt, kb, e0))
                            at = atpool.tile([128, 1024], bf16, tag="at",
                                             name="at")
                            e0L = info[0][2]
                            nc.scalar.activation(
                                at[:, e0L:1024], Ps[:, e0L:1024], EXP,
                                scale=0.125)
                            for sub in range(2):
                                jt, kb, e0 = info[sub]
                                if kb >= 0:
                                    nc.gpsimd.affine_select(
                                        out=at[:, sub * 512 + e0:
                                               sub * 512 + e0 + 128],
                                        in_=at[:, sub * 512 + e0:
                                               sub * 512 + e0 + 128],
                                        compare_op=mybir.AluOpType.is_ge,
                                        fill=0.0, base=0, pattern=[[1, 128]],
                                        channel_multiplier=-1)
                            flush()
                            for sub in range(2):
                                jt, kb, e0 = info[sub]
                                pend.append((at, jt, e0, sub))
                            state["pcount"] += 1
                            if bg and state["pcount"] % stride == 0:
                                bg.pop(0)()
                        flush()
                        nc.vector.tensor_copy(den[h * 32:h * 32 + 1, :],
                                              Pc[64:65, :])
                        nc.vector.tensor_copy(
                            ctxT[h * 64:(h + 1) * 64, :], Pc[0:64, :])
                    scr = smpool.tile([33, 512], f32, tag="scr", name="scr")
                    rr = smpool.tile([33, 512], f32, tag="rr", name="rr")
                    nc.vector.reciprocal_approx_accurate(rr[:], den[:], scr[:])
                    rhi = smpool.tile([33, 512], bf16, tag="rhi", name="rhi")
                    nc.vector.tensor_copy(rhi[:], rr[:])
                    rlo = smpool.tile([33, 512], bf16, tag="rlo", name="rlo")
                    nc.vector.tensor_sub(rlo[:], rr[:], rhi[:])
                    chains.append((ctxT, rhi, rlo))
                # leftover background groups
                while bg:
                    bg.pop(0)()
                # normalize: cx = ctxT * (1/den) broadcast via E matmul
                for hp in range(2):
                    ctxT, rhi, rlo = chains[hp]
                    Pb = ps_m.tile([128, 512], f32, tag="m", name="Pb")
                    nc.tensor.matmul(Pb[:], emat[:], rhi[:], start=True,
                                     stop=False, skip_group_check=True)
                    nc.tensor.matmul(Pb[:], emat[:], rlo[:], start=False,
                                     stop=True, skip_group_check=True)
                    cx = cxpool.tile([128, 512], bf16, tag=f"cx{hp}",
                                     name=f"cx{hp}")
                    nc.vector.tensor_mul(cx[:], ctxT[:], Pb[:])
                    cxs.append(cx)
                return cxs

            # ---- main schedule ----
            for g in proj_groups(0):
                g()
            prev_cxs = None
            for jc in range(IC):
                bg = []
                if jc + 1 < IC:
                    bg += proj_groups(jc + 1)
                if prev_cxs is not None:
                    bg += outproj_units(jc - 1, prev_cxs)
                prev_cxs = attention(jc, bg)
            for u in outproj_units(IC - 1, prev_cxs):
                u()

    nc.compile()
    return nc


def _get_nc():
    if "nc" not in _CACHE:
        _install_ntff_hook()
        _CACHE["nc"] = _build()
    return _CACHE["nc"]


def _run(inputs, trace=False):
    from concourse.bass_utils import run_bass_kernel_spmd
    import ml_dtypes

    nc = _get_nc()
    x = np.asarray(inputs["x"], dtype=np.float32)
    Wq = np.asarray(inputs["Wq"], dtype=np.float32)
    Wk = np.asarray(inputs["Wk"], dtype=np.float32)
    Wv = np.asarray(inputs["Wv"], dtype=np.float32)
    Wo = np.asarray(inputs["Wo"], dtype=np.float32)
    bo = np.asarray(inputs["bo"], dtype=np.float32)
    bf = ml_dtypes.bfloat16

    def kt_major(a):
        # [1024, 256] -> [128, KT*256] kt-major per partition
        return np.ascontiguousarray(
            a.reshape(KT, 128, DLOC).transpose(1, 0, 2)
            .reshape(128, KT * DLOC)).astype(bf)

    xts = []
    for b in range(B):
        # [1024, 2048] -> [128, jc*4096 + kt*512 + r]
        xtb = np.ascontiguousarray(x[b].T)
        xtb = xtb.reshape(KT, 128, IC, 512).transpose(1, 2, 0, 3)
        xts.append(np.ascontiguousarray(xtb.reshape(128, KT * SB)).astype(bf))

    in_maps = []
    for c in range(NCORES):
        b, hg = divmod(c, 4)
        sl = slice(hg * DLOC, (hg + 1) * DLOC)
        in_maps.append({
            "xt": xts[b],
            "wq": kt_major(Wq[:, sl]),
            "wk": kt_major(Wk[:, sl]),
            "wv": kt_major(Wv[:, sl]),
            "wo": np.ascontiguousarray(
                Wo[sl, :].reshape(2, 128, D).transpose(1, 0, 2)
                .reshape(128, 2 * D)).astype(bf),
        })
    res = run_bass_kernel_spmd(nc, in_maps, core_ids=list(range(NCORES)),
                               trace=trace)
    out = np.zeros((B, SB, D), dtype=np.float32)
    for c in range(NCORES):
        b = c // 4
        out[b] += res.results[c]["out"].astype(np.float32)
    out += bo[None, None, :]
    return out, res


def kernel(**inputs):
    out, _ = _run(inputs, trace=False)
    return out
